# revision 5
# baseline (speedup 1.0000x reference)
"""GraphSAGE/GraphConv (DGL norm='both') Bass kernel for 8 Trainium2 cores.

Math (reference):
  x[n,f]   : node features, n in [0,160000), f in [0,64)   (from inputs[8,64,20000])
  agg[d]   = norm_dst[d] * sum_{e: dst[e]=d} norm_src[src[e]] * x[src[e]]
  out      = leaky_relu(agg @ W + b, 0.01), returned as [8,64,20000] feature-major.

Device strategy (per core, vertex-cut on dst):
  - core c owns dst nodes [c*20000,(c+1)*20000) == output slice c of dim 0.
  - norm_src is folded into the x table on the host (x_tab[n] = x[n]*norm_src[n]),
    norm_dst is applied per dst node at the ZT evacuation (leaky-relu is
    positive-homogeneous, so the scale commutes through agg@W; bias handled
    before the activation).
  - edges bucketed by (dst tile of TW nodes, src window of WCLS rows), padded
    to 128-edge chunks.  Bulk dma_gather (int16 window-local indices, 256B fp32
    rows) pulls x[src] rows edge-major; descriptor generation is spread over
    4 SWDGE queues (distinct Q7 cpu pairs) so it runs ~3x parallel.
  - ACT casts gathered rows fp32->bf16; DVE builds the one-hot
    S01[e,d]=(dst_local[e]==d); TensorE accumulates aggT[f,d] += G[e,:]^T @ S01
    in PSUM per dst tile.
  - aggT evac (ACT, ->bf16), Z[d,o] = aggT[:,half]^T @ W per 128-row half
    (node-major!), evac with per-partition norm_dst scale + Lrelu (ACT),
    batched output DMA.  Host transposes [20000,64] -> [64,20000] per core.

Host does index/layout work plus the degree-normalization folds; all per-edge
feature math runs on device.
"""

import os

import numpy as np
import ml_dtypes

from concourse import bass, mybir
import concourse.bacc as bacc
from concourse.tile import TileContext
from concourse.bass_utils import run_bass_kernel_spmd

BF16 = ml_dtypes.bfloat16
F32 = np.float32

LAST_RESULTS = None  # test harness introspection (exec time / trace)

CHUNK = 128     # edges per matmul chunk (PE contraction dim)
TW = 128        # dst-tile width (one-hot columns / PSUM free dim)
WCLS = 32768    # src index window (int16 range for dma_gather)
MAX_RUN = 8     # chunks per dma_gather instruction (ring limit ~1024 idxs)
NQ = 4          # SWDGE queues (Q7 cpu pairs) to round-robin gathers over
STAGE_SLOTS = 8  # 128-row output halves per staged write-back
DLPAD = 300.0   # one-hot miss value for padded slots (> TW-1)


def _build_layout(src, dst, n_nodes, n_cores, npc, wcls, tw):
    """Tile-major slot layout: chunk m, partition p; arrays [128, M] per core,
    one shared shape.  For each dst tile t, runs of per-(t, r) segments
    (r = src window), each run <= MAX_RUN chunks.  Within each segment edges
    are sorted by src (better DRAM locality).
    """
    nt = -(-npc // tw)
    ncls = -(-n_nodes // wcls)

    owner = (dst // npc).astype(np.int64)
    rem = dst - owner * npc
    tile = rem // tw
    dl = rem - tile * tw
    r = (src // wcls).astype(np.int64)
    src_local = (src - r * wcls).astype(np.int16)

    # segment key: (core, tile, class); edges sorted by key then src
    key = (owner * nt + tile) * ncls + r
    order = np.lexsort((src, key))
    nseg = n_cores * nt * ncls
    counts = np.bincount(key, minlength=nseg).reshape(n_cores, nt, ncls)

    # shared chunk counts: max over cores
    chunks_tr = -(-counts.max(axis=0) // CHUNK)  # [nt, ncls]
    chunks_tr[0, 0] = max(chunks_tr[0, 0], 1)

    flat = chunks_tr.reshape(-1)
    seg_chunk_start = np.zeros(nt * ncls + 1, np.int64)
    np.cumsum(flat, out=seg_chunk_start[1:])
    M = int(seg_chunk_start[-1])

    # runs per tile: (a, b, r) chunk ranges, split at MAX_RUN
    tile_runs = []
    for t in range(nt):
        runs = []
        for rr in range(ncls):
            s = int(seg_chunk_start[t * ncls + rr])
            n = int(chunks_tr[t, rr])
            a = s
            while a < s + n:
                b = min(a + MAX_RUN, s + n)
                runs.append((a, b, rr))
                a = b
        tile_runs.append(runs)

    # slot fill (vectorized over all edges)
    seg_start = np.zeros(nseg + 1, np.int64)
    np.cumsum(counts.reshape(-1), out=seg_start[1:])
    key_s = key[order]
    pos = np.arange(len(order), dtype=np.int64) - seg_start[key_s]
    tr_key = key_s % (nt * ncls)            # (t, r) within core
    c_s = key_s // (nt * ncls)
    m_s = seg_chunk_start[tr_key] + pos // CHUNK
    p_s = pos - (pos // CHUNK) * CHUNK

    idx_slot = np.zeros((n_cores, 128, M), np.int16)
    dl_all = np.full((n_cores, 128, M), DLPAD, F32)
    idx_slot[c_s, p_s, m_s] = src_local[order]
    dl_all[c_s, p_s, m_s] = dl[order].astype(F32)

    # dma_gather index stream: flat order i = chunk-major (k*128+p), wrapped
    # into 16 partitions (idx16[j, s] = flat[s*16+j]) and replicated x8.
    idx16 = np.zeros((n_cores, 128, M * (CHUNK // 16)), np.int16)
    for c in range(n_cores):
        flat_i = idx_slot[c].T.reshape(-1)            # [M*128], chunk-major
        wrapped = flat_i.reshape(-1, 16).T            # [16, M*8]
        idx16[c] = np.tile(wrapped, (8, 1))
    return idx16, dl_all, tile_runs, M, nt, ncls


def _build_nc(n_nodes, feat, outd, M, nt, npc, tile_runs, n_cores, wcls,
              bias_is_zero):
    f32 = mybir.dt.float32
    bf16 = mybir.dt.bfloat16
    i16 = mybir.dt.int16

    nc = bacc.Bacc(
        "TRN2",
        target_bir_lowering=False,
        debug=False,
        enable_asserts=False,
        num_devices=n_cores,
        num_swdge_queues=NQ,
    )

    nhalf = -(-npc // 128)
    scols = M * (CHUNK // 16)
    x_t = nc.dram_tensor("x_tab", [n_nodes, feat], f32, kind="ExternalInput")
    idx_t = nc.dram_tensor("idx16", [128, scols], i16, kind="ExternalInput")
    dl_t = nc.dram_tensor("dl", [128, M], f32, kind="ExternalInput")
    iota_t = nc.dram_tensor("iota", [128, TW], bf16, kind="ExternalInput")
    W_t = nc.dram_tensor("Wt", [feat, outd], bf16, kind="ExternalInput")
    brep_t = nc.dram_tensor("brep", [128, outd], f32, kind="ExternalInput")
    ndst_t = nc.dram_tensor("ndst", [128, nhalf], f32, kind="ExternalInput")
    out_t = nc.dram_tensor("out", [npc, outd], f32, kind="ExternalOutput")

    with TileContext(nc) as tc:
        with (
            tc.tile_pool(name="const", bufs=1) as constp,
            tc.tile_pool(name="gbuf", bufs=12) as gpool,
            tc.tile_pool(name="cast", bufs=6) as cpool,
            tc.tile_pool(name="onehot", bufs=6) as spool,
            tc.tile_pool(name="evac", bufs=3) as evacp,
            tc.tile_pool(name="stage", bufs=2) as stagep,
            tc.tile_pool(name="lk", bufs=2) as lkp,
            tc.tile_pool(name="psA", bufs=3, space="PSUM") as psA,
            tc.tile_pool(name="psZ", bufs=2, space="PSUM") as psZ,
        ):
            idx_sb = constp.tile([128, scols], i16)
            nc.sync.dma_start(idx_sb[:], idx_t[:])
            dl_sb = constp.tile([128, M], f32)
            nc.sync.dma_start(dl_sb[:], dl_t[:])
            iota_sb = constp.tile([128, TW], bf16)
            nc.sync.dma_start(iota_sb[:], iota_t[:])
            W_sb = constp.tile([feat, outd], bf16)
            nc.sync.dma_start(W_sb[:], W_t[:])
            brep_sb = constp.tile([128, outd], f32)
            nc.sync.dma_start(brep_sb[:], brep_t[:])
            ndst_sb = constp.tile([128, nhalf], f32)
            nc.sync.dma_start(ndst_sb[:], ndst_t[:])

            for _ in range(12):
                g0 = gpool.tile([128, MAX_RUN * feat], f32, tag="g")
                nc.vector.memset(g0[:], 0.0)

            gq = 0            # gather instruction counter (queue round-robin)
            stage = None
            stage_fill = 0    # half-slots currently staged
            stage_base = 0    # first half index in the stage
            lrelu = mybir.ActivationFunctionType.Lrelu
            copyf = mybir.ActivationFunctionType.Copy

            def flush_stage():
                nonlocal stage, stage_fill, stage_base
                if stage is None or stage_fill == 0:
                    return
                ns = stage_fill
                span = stage[:, : ns * outd]
                if not bias_is_zero:
                    nc.vector.tensor_tensor(
                        out=span.rearrange("p (k f) -> p k f", f=outd),
                        in0=span.rearrange("p (k f) -> p k f", f=outd),
                        in1=brep_sb[:]
                        .rearrange("p (o f) -> p o f", o=1)
                        .to_broadcast([128, ns, outd]),
                        op=mybir.AluOpType.add,
                    )
                    zs = lkp.tile([128, STAGE_SLOTS * outd], f32, tag="zs")
                    nc.vector.tensor_scalar(
                        out=zs[:, : ns * outd],
                        in0=span,
                        scalar1=0.01,
                        scalar2=None,
                        op0=mybir.AluOpType.mult,
                    )
                    nc.vector.tensor_tensor(
                        out=span,
                        in0=span,
                        in1=zs[:, : ns * outd],
                        op=mybir.AluOpType.max,
                    )
                row0 = stage_base * 128
                nfull = min(ns, (npc - row0) // 128)
                if nfull > 0:
                    nc.sync.dma_start(
                        out_t[row0 : row0 + nfull * 128, :].rearrange(
                            "(t p) f -> p t f", p=128
                        ),
                        stage[:, : nfull * outd].rearrange(
                            "p (t f) -> p t f", f=outd
                        ),
                    )
                if nfull < ns:  # partial last 128-block (tail of the core)
                    rr0 = row0 + nfull * 128
                    nv = npc - rr0
                    nc.sync.dma_start(
                        out_t[rr0:npc, :],
                        stage[:nv, nfull * outd : (nfull + 1) * outd],
                    )
                stage = None
                stage_fill = 0

            for t in range(nt):
                runs = tile_runs[t]
                nch = sum(b - a for a, b, _ in runs)
                aggT = psA.tile([feat, TW], f32, tag="agg")
                j = 0
                for a, b, rr in runs:
                    K = b - a
                    row0 = rr * wcls
                    row1 = min(n_nodes, (rr + 1) * wcls)
                    gt = gpool.tile([128, MAX_RUN * feat], f32, tag="g")
                    nc.gpsimd.dma_gather(
                        out_ap=gt[:, : K * feat].rearrange(
                            "p (k f) -> p k f", f=feat
                        ),
                        in_ap=x_t[row0:row1, :],
                        idxs_ap=idx_sb[:, a * (CHUNK // 16) : b * (CHUNK // 16)],
                        num_idxs=K * CHUNK,
                        num_idxs_reg=K * CHUNK,
                        elem_size=feat,
                        queue_num=gq % NQ,
                    )
                    gq += 1
                    ct = cpool.tile([128, MAX_RUN * feat], bf16, tag="c")
                    nc.scalar.activation(ct[:, : K * feat], gt[:, : K * feat], copyf)
                    st = spool.tile([128, MAX_RUN * TW], bf16, tag="s")
                    for col in range(K):
                        nc.vector.tensor_scalar(
                            out=st[:, col * TW : (col + 1) * TW],
                            in0=iota_sb[:],
                            scalar1=dl_sb[:, a + col : a + col + 1],
                            scalar2=None,
                            op0=mybir.AluOpType.is_equal,
                        )
                    for col in range(K):
                        nc.tensor.matmul(
                            out=aggT[:],
                            lhsT=ct[:, col * feat : (col + 1) * feat],
                            rhs=st[:, col * TW : (col + 1) * TW],
                            start=(j == 0),
                            stop=(j == nch - 1),
                        )
                        j += 1
                # aggT -> SBUF bf16, then per-128 half: Z[d,o], scale, stage
                aggs = evacp.tile([feat, TW], bf16, tag="ev")
                nc.scalar.activation(aggs[:], aggT[:], copyf)
                for h in range(TW // 128):
                    half = t * (TW // 128) + h
                    if half * 128 >= npc:
                        continue
                    if stage is None:
                        stage = stagep.tile([128, STAGE_SLOTS * outd], f32, tag="st")
                        stage_base = half
                    Z = psZ.tile([128, outd], f32, tag="z")
                    nc.tensor.matmul(
                        out=Z[:],
                        lhsT=aggs[:, h * 128 : (h + 1) * 128],
                        rhs=W_sb[:],
                        start=True,
                        stop=True,
                    )
                    sl = stage_fill
                    nc.scalar.activation(
                        stage[:, sl * outd : (sl + 1) * outd],
                        Z[:],
                        lrelu if bias_is_zero else copyf,
                        scale=ndst_sb[:, half : half + 1],
                        alpha=0.01,
                    )
                    stage_fill += 1
                    if stage_fill == STAGE_SLOTS:
                        flush_stage()
            flush_stage()

    nc.compile()
    return nc


def _prep(inputs, W, b, src, dst, n_cores, wcls=WCLS):
    sli, feat, node = inputs.shape
    n_nodes = sli * node
    outd = W.shape[1]
    npc = n_nodes // n_cores

    src = np.asarray(src).astype(np.int64)
    dst = np.asarray(dst).astype(np.int64)
    deg_out = np.bincount(src, minlength=n_nodes)
    deg_in = np.bincount(dst, minlength=n_nodes)
    norm_src = np.maximum(deg_out, 1).astype(F32) ** -0.5
    norm_dst = np.maximum(deg_in, 1).astype(F32) ** -0.5

    # node-major feature table with norm_src pre-folded
    x_tab = np.ascontiguousarray(
        np.asarray(inputs, dtype=F32).transpose(0, 2, 1).reshape(n_nodes, feat)
    ) * norm_src[:, None]
    x_tab = np.ascontiguousarray(x_tab, dtype=F32)

    idx16, dl_all, tile_runs, M, nt, ncls = _build_layout(
        src, dst, n_nodes, n_cores, npc, wcls, TW
    )

    nhalf = -(-npc // 128)
    iota = np.broadcast_to(np.arange(TW, dtype=F32), (128, TW)).astype(BF16)
    Wt = np.asarray(W, dtype=F32).astype(BF16)
    b_arr = np.asarray(b, dtype=F32).reshape(-1)
    brep = np.broadcast_to(b_arr, (128, outd)).astype(F32)
    ndst = np.ones((n_cores, 128, nhalf), F32)
    for c in range(n_cores):
        base = c * npc
        cols = np.arange(nhalf) * 128
        idxg = base + cols[None, :] + np.arange(128)[:, None]
        valid = idxg < base + npc
        ndst[c][valid] = norm_dst[idxg[valid]]

    in_maps = []
    for c in range(n_cores):
        in_maps.append(
            {
                "x_tab": x_tab,
                "idx16": np.ascontiguousarray(idx16[c]),
                "dl": np.ascontiguousarray(dl_all[c]),
                "iota": np.ascontiguousarray(iota),
                "Wt": Wt,
                "brep": np.ascontiguousarray(brep),
                "ndst": np.ascontiguousarray(ndst[c]),
            }
        )
    meta = dict(
        n_nodes=n_nodes, feat=feat, outd=outd, M=M, nt=nt, npc=npc,
        tile_runs=tile_runs, sli=sli, node=node, wcls=wcls,
        bias_is_zero=bool(np.all(b_arr == 0.0)),
    )
    return in_maps, meta


def kernel(inputs, W, b, src, dst):
    global LAST_RESULTS
    n_cores = 8
    inputs = np.asarray(inputs, dtype=F32)
    in_maps, meta = _prep(inputs, W, b, src, dst, n_cores)

    nc = _build_nc(
        meta["n_nodes"], meta["feat"], meta["outd"], meta["M"], meta["nt"],
        meta["npc"], meta["tile_runs"], n_cores, meta["wcls"],
        meta["bias_is_zero"],
    )

    res = run_bass_kernel_spmd(
        nc,
        in_maps,
        core_ids=list(range(n_cores)),
        trace=bool(int(os.environ.get("KERNEL_TRACE", "0"))),
    )
    LAST_RESULTS = res

    # device output is node-major [npc, outd]; transpose to [outd, npc]
    out = np.stack(
        [r["out"].T for r in res.results], axis=0
    )  # [8, 64, 20000]
    return np.ascontiguousarray(out, dtype=F32)


# revision 7
# speedup vs baseline: 1.7603x; 1.7603x over previous
"""GraphSAGE/GraphConv (DGL norm='both') Bass kernel for 8 Trainium2 cores.

Math (reference):
  x[n,f]   : node features, n in [0,160000), f in [0,64)   (from inputs[8,64,20000])
  agg[d]   = norm_dst[d] * sum_{e: dst[e]=d} norm_src[src[e]] * x[src[e]]
  out      = leaky_relu(agg @ W + b, 0.01), returned as [8,64,20000] feature-major.

Device strategy (per core, vertex-cut on dst):
  - core c owns dst nodes [c*20000,(c+1)*20000) == output slice c of dim 0.
  - norm_src is folded into the x table on the host (x_tab[n] = x[n]*norm_src[n]),
    norm_dst is applied per dst node at the ZT evacuation (leaky-relu is
    positive-homogeneous, so the scale commutes through agg@W; bias handled
    before the activation).
  - edges bucketed by (dst tile of TW nodes, src window of WCLS rows), padded
    to 128-edge chunks.  Bulk dma_gather (int16 window-local indices, 256B fp32
    rows) pulls x[src] rows edge-major; descriptor generation is spread over
    4 SWDGE queues (distinct Q7 cpu pairs) so it runs ~3x parallel.
  - ACT casts gathered rows fp32->bf16; DVE builds the one-hot
    S01[e,d]=(dst_local[e]==d); TensorE accumulates aggT[f,d] += G[e,:]^T @ S01
    in PSUM per dst tile.
  - aggT evac (ACT, ->bf16), Z[d,o] = aggT[:,half]^T @ W per 128-row half
    (node-major!), evac with per-partition norm_dst scale + Lrelu (ACT),
    batched output DMA.  Host transposes [20000,64] -> [64,20000] per core.

Host does index/layout work plus the degree-normalization folds; all per-edge
feature math runs on device.
"""

import os

import numpy as np
import ml_dtypes

from concourse import bass, mybir
import concourse.bacc as bacc
from concourse.tile import TileContext
from concourse.bass_utils import run_bass_kernel_spmd

BF16 = ml_dtypes.bfloat16
F32 = np.float32

LAST_RESULTS = None  # test harness introspection (exec time / trace)

CHUNK = 128     # edges per matmul chunk (PE contraction dim)
TW = 256        # dst-tile width (one-hot columns / PSUM free dim)
WCLS = 32768    # src index window (int16 range for dma_gather)
MAX_RUN = 8     # chunks per dma_gather instruction (ring limit ~1024 idxs)
NQ = 4          # SWDGE queues (Q7 cpu pairs) to round-robin gathers over
STAGE_SLOTS = 8  # 128-row output halves per staged write-back
DLPAD = 300.0   # one-hot miss value for padded slots (> TW-1)


def _build_layout(src, dst, n_nodes, n_cores, npc, wcls, tw):
    """Tile-major slot layout: chunk m, partition p; arrays [128, M] per core,
    one shared shape.  For each dst tile t, runs of per-(t, r) segments
    (r = src window), each run <= MAX_RUN chunks.  Within each segment edges
    are sorted by src (better DRAM locality).
    """
    nt = -(-npc // tw)
    ncls = -(-n_nodes // wcls)

    owner = (dst // npc).astype(np.int64)
    rem = dst - owner * npc
    tile = rem // tw
    dl = rem - tile * tw
    r = (src // wcls).astype(np.int64)
    src_local = (src - r * wcls).astype(np.int16)

    # segment key: (core, tile, class); edges sorted by key then src
    key = (owner * nt + tile) * ncls + r
    order = np.lexsort((src, key))
    nseg = n_cores * nt * ncls
    counts = np.bincount(key, minlength=nseg).reshape(n_cores, nt, ncls)

    # shared chunk counts: max over cores
    chunks_tr = -(-counts.max(axis=0) // CHUNK)  # [nt, ncls]
    chunks_tr[0, 0] = max(chunks_tr[0, 0], 1)

    flat = chunks_tr.reshape(-1)
    seg_chunk_start = np.zeros(nt * ncls + 1, np.int64)
    np.cumsum(flat, out=seg_chunk_start[1:])
    M = int(seg_chunk_start[-1])

    # runs per tile: (a, b, r) chunk ranges, split at MAX_RUN
    tile_runs = []
    for t in range(nt):
        runs = []
        for rr in range(ncls):
            s = int(seg_chunk_start[t * ncls + rr])
            n = int(chunks_tr[t, rr])
            a = s
            while a < s + n:
                b = min(a + MAX_RUN, s + n)
                runs.append((a, b, rr))
                a = b
        tile_runs.append(runs)

    # slot fill (vectorized over all edges)
    seg_start = np.zeros(nseg + 1, np.int64)
    np.cumsum(counts.reshape(-1), out=seg_start[1:])
    key_s = key[order]
    pos = np.arange(len(order), dtype=np.int64) - seg_start[key_s]
    tr_key = key_s % (nt * ncls)            # (t, r) within core
    c_s = key_s // (nt * ncls)
    m_s = seg_chunk_start[tr_key] + pos // CHUNK
    p_s = pos - (pos // CHUNK) * CHUNK

    idx_slot = np.full((n_cores, 128, M), -1, np.int16)
    dl_all = np.full((n_cores, 128, M), DLPAD, BF16)
    idx_slot[c_s, p_s, m_s] = src_local[order]
    dl_all[c_s, p_s, m_s] = dl[order].astype(BF16)

    # per-core valid idx count per run (for num_idxs_reg; trailing -1s are
    # trimmed by the Q7 so descriptor work is exact per core)
    run_cnt = []
    for t in range(nt):
        for a, b, rr in tile_runs[t]:
            s0 = int(seg_chunk_start[t * ncls + rr])
            base = (a - s0) * CHUNK
            cnt = np.clip(counts[:, t, rr] - base, 0, (b - a) * CHUNK)
            run_cnt.append(cnt)
    run_cnt = np.stack(run_cnt, axis=1).astype(np.int32)  # [n_cores, ninstr]

    # dma_gather index stream: flat order i = chunk-major (k*128+p), wrapped
    # into 16 partitions (idx16[j, s] = flat[s*16+j]) and replicated x8.
    idx16 = np.zeros((n_cores, 128, M * (CHUNK // 16)), np.int16)
    for c in range(n_cores):
        flat_i = idx_slot[c].T.reshape(-1)            # [M*128], chunk-major
        wrapped = flat_i.reshape(-1, 16).T            # [16, M*8]
        idx16[c] = np.tile(wrapped, (8, 1))
    return idx16, dl_all, tile_runs, run_cnt, M, nt, ncls


def _build_nc(n_nodes, feat, outd, M, nt, npc, tile_runs, n_cores, wcls,
              bias_is_zero, ninstr):
    f32 = mybir.dt.float32
    i32 = mybir.dt.int32
    bf16 = mybir.dt.bfloat16
    i16 = mybir.dt.int16

    nc = bacc.Bacc(
        "TRN2",
        target_bir_lowering=False,
        debug=False,
        enable_asserts=False,
        num_devices=n_cores,
        num_swdge_queues=NQ,
    )

    nhalf = -(-npc // 128)
    scols = M * (CHUNK // 16)
    x_t = nc.dram_tensor("x_tab", [n_nodes, feat], f32, kind="ExternalInput")
    idx_t = nc.dram_tensor("idx16", [128, scols], i16, kind="ExternalInput")
    dl_t = nc.dram_tensor("dl", [128, M], bf16, kind="ExternalInput")
    iota_t = nc.dram_tensor("iota", [128, TW], bf16, kind="ExternalInput")
    W_t = nc.dram_tensor("Wt", [feat, outd], bf16, kind="ExternalInput")
    brep_t = nc.dram_tensor("brep", [128, outd], f32, kind="ExternalInput")
    ndst_t = nc.dram_tensor("ndst", [128, nhalf], f32, kind="ExternalInput")
    gcnt_t = nc.dram_tensor("gcnt", [1, ninstr], i32, kind="ExternalInput")
    out_t = nc.dram_tensor("out", [npc, outd], f32, kind="ExternalOutput")

    with TileContext(nc) as tc:
        with (
            tc.tile_pool(name="const", bufs=1) as constp,
            tc.tile_pool(name="gbuf", bufs=12) as gpool,
            tc.tile_pool(name="cast", bufs=6) as cpool,
            tc.tile_pool(name="onehot", bufs=6) as spool,
            tc.tile_pool(name="evac", bufs=3) as evacp,
            tc.tile_pool(name="stage", bufs=2) as stagep,
            tc.tile_pool(name="lk", bufs=2) as lkp,
            tc.tile_pool(name="psA", bufs=3, space="PSUM") as psA,
            tc.tile_pool(name="psZ", bufs=2, space="PSUM") as psZ,
        ):
            idx_sb = constp.tile([128, scols], i16)
            nc.sync.dma_start(idx_sb[:], idx_t[:])
            dl_sb = constp.tile([128, M], bf16)
            nc.sync.dma_start(dl_sb[:], dl_t[:])
            iota_sb = constp.tile([128, TW], bf16)
            nc.sync.dma_start(iota_sb[:], iota_t[:])
            W_sb = constp.tile([feat, outd], bf16)
            nc.sync.dma_start(W_sb[:], W_t[:])
            brep_sb = constp.tile([128, outd], f32)
            nc.sync.dma_start(brep_sb[:], brep_t[:])
            ndst_sb = constp.tile([128, nhalf], f32)
            nc.sync.dma_start(ndst_sb[:], ndst_t[:])
            gcnt_sb = constp.tile([1, ninstr], i32)
            nc.sync.dma_start(gcnt_sb[:], gcnt_t[:])
            cnt_reg = nc.gpsimd.alloc_register("gcnt_reg")

            for _ in range(12):
                g0 = gpool.tile([128, MAX_RUN * feat], f32, tag="g")
                nc.vector.memset(g0[:], 0.0)

            gq = 0            # gather instruction counter (queue round-robin)
            stage = None
            stage_fill = 0    # half-slots currently staged
            stage_base = 0    # first half index in the stage
            lrelu = mybir.ActivationFunctionType.Lrelu
            copyf = mybir.ActivationFunctionType.Copy

            def flush_stage():
                nonlocal stage, stage_fill, stage_base
                if stage is None or stage_fill == 0:
                    return
                ns = stage_fill
                span = stage[:, : ns * outd]
                if not bias_is_zero:
                    nc.vector.tensor_tensor(
                        out=span.rearrange("p (k f) -> p k f", f=outd),
                        in0=span.rearrange("p (k f) -> p k f", f=outd),
                        in1=brep_sb[:]
                        .rearrange("p (o f) -> p o f", o=1)
                        .to_broadcast([128, ns, outd]),
                        op=mybir.AluOpType.add,
                    )
                    zs = lkp.tile([128, STAGE_SLOTS * outd], f32, tag="zs")
                    nc.vector.tensor_scalar(
                        out=zs[:, : ns * outd],
                        in0=span,
                        scalar1=0.01,
                        scalar2=None,
                        op0=mybir.AluOpType.mult,
                    )
                    nc.vector.tensor_tensor(
                        out=span,
                        in0=span,
                        in1=zs[:, : ns * outd],
                        op=mybir.AluOpType.max,
                    )
                row0 = stage_base * 128
                nfull = min(ns, (npc - row0) // 128)
                if nfull > 0:
                    nc.sync.dma_start(
                        out_t[row0 : row0 + nfull * 128, :].rearrange(
                            "(t p) f -> p t f", p=128
                        ),
                        stage[:, : nfull * outd].rearrange(
                            "p (t f) -> p t f", f=outd
                        ),
                    )
                if nfull < ns:  # partial last 128-block (tail of the core)
                    rr0 = row0 + nfull * 128
                    nv = npc - rr0
                    nc.sync.dma_start(
                        out_t[rr0:npc, :],
                        stage[:nv, nfull * outd : (nfull + 1) * outd],
                    )
                stage = None
                stage_fill = 0

            for t in range(nt):
                runs = tile_runs[t]
                nch = sum(b - a for a, b, _ in runs)
                aggT = psA.tile([feat, TW], f32, tag="agg")
                j = 0
                for a, b, rr in runs:
                    K = b - a
                    row0 = rr * wcls
                    row1 = min(n_nodes, (rr + 1) * wcls)
                    gt = gpool.tile([128, MAX_RUN * feat], f32, tag="g")
                    nc.gpsimd.reg_load(cnt_reg, gcnt_sb[0:1, gq : gq + 1])
                    nc.gpsimd.dma_gather(
                        out_ap=gt[:, : K * feat].rearrange(
                            "p (k f) -> p k f", f=feat
                        ),
                        in_ap=x_t[row0:row1, :],
                        idxs_ap=idx_sb[:, a * (CHUNK // 16) : b * (CHUNK // 16)],
                        num_idxs=K * CHUNK,
                        num_idxs_reg=cnt_reg,
                        elem_size=feat,
                        queue_num=gq % NQ,
                    )
                    gq += 1
                    ct = cpool.tile([128, MAX_RUN * feat], bf16, tag="c")
                    nc.scalar.activation(ct[:, : K * feat], gt[:, : K * feat], copyf)
                    st = spool.tile([128, MAX_RUN * TW], bf16, tag="s")
                    nc.vector.tensor_tensor(
                        out=st[:, : K * TW].rearrange("p (k d) -> p k d", d=TW),
                        in0=iota_sb[:]
                        .rearrange("p (o d) -> p o d", o=1)
                        .to_broadcast([128, K, TW]),
                        in1=dl_sb[:, a:b]
                        .rearrange("p (k o) -> p k o", o=1)
                        .to_broadcast([128, K, TW]),
                        op=mybir.AluOpType.is_equal,
                    )
                    for col in range(K):
                        nc.tensor.matmul(
                            out=aggT[:],
                            lhsT=ct[:, col * feat : (col + 1) * feat],
                            rhs=st[:, col * TW : (col + 1) * TW],
                            start=(j == 0),
                            stop=(j == nch - 1),
                        )
                        j += 1
                # aggT -> SBUF bf16, then per-128 half: Z[d,o], scale, stage
                aggs = evacp.tile([feat, TW], bf16, tag="ev")
                nc.scalar.activation(aggs[:], aggT[:], copyf)
                for h in range(TW // 128):
                    half = t * (TW // 128) + h
                    if half * 128 >= npc:
                        continue
                    if stage is None:
                        stage = stagep.tile([128, STAGE_SLOTS * outd], f32, tag="st")
                        stage_base = half
                    Z = psZ.tile([128, outd], f32, tag="z")
                    nc.tensor.matmul(
                        out=Z[:],
                        lhsT=aggs[:, h * 128 : (h + 1) * 128],
                        rhs=W_sb[:],
                        start=True,
                        stop=True,
                    )
                    sl = stage_fill
                    nc.scalar.activation(
                        stage[:, sl * outd : (sl + 1) * outd],
                        Z[:],
                        lrelu if bias_is_zero else copyf,
                        scale=ndst_sb[:, half : half + 1],
                        alpha=0.01,
                    )
                    stage_fill += 1
                    if stage_fill == STAGE_SLOTS:
                        flush_stage()
            flush_stage()

    nc.compile()
    return nc


def _prep(inputs, W, b, src, dst, n_cores, wcls=WCLS):
    sli, feat, node = inputs.shape
    n_nodes = sli * node
    outd = W.shape[1]
    npc = n_nodes // n_cores

    src = np.asarray(src).astype(np.int64)
    dst = np.asarray(dst).astype(np.int64)
    deg_out = np.bincount(src, minlength=n_nodes)
    deg_in = np.bincount(dst, minlength=n_nodes)
    norm_src = np.maximum(deg_out, 1).astype(F32) ** -0.5
    norm_dst = np.maximum(deg_in, 1).astype(F32) ** -0.5

    # node-major feature table with norm_src pre-folded
    x_tab = np.ascontiguousarray(
        np.asarray(inputs, dtype=F32).transpose(0, 2, 1).reshape(n_nodes, feat)
    ) * norm_src[:, None]
    x_tab = np.ascontiguousarray(x_tab, dtype=F32)

    idx16, dl_all, tile_runs, run_cnt, M, nt, ncls = _build_layout(
        src, dst, n_nodes, n_cores, npc, wcls, TW
    )

    nhalf = -(-npc // 128)
    iota = np.broadcast_to(np.arange(TW, dtype=F32), (128, TW)).astype(BF16)
    Wt = np.asarray(W, dtype=F32).astype(BF16)
    b_arr = np.asarray(b, dtype=F32).reshape(-1)
    brep = np.broadcast_to(b_arr, (128, outd)).astype(F32)
    ndst = np.ones((n_cores, 128, nhalf), F32)
    for c in range(n_cores):
        base = c * npc
        cols = np.arange(nhalf) * 128
        idxg = base + cols[None, :] + np.arange(128)[:, None]
        valid = idxg < base + npc
        ndst[c][valid] = norm_dst[idxg[valid]]

    in_maps = []
    for c in range(n_cores):
        in_maps.append(
            {
                "x_tab": x_tab,
                "idx16": np.ascontiguousarray(idx16[c]),
                "dl": np.ascontiguousarray(dl_all[c]),
                "iota": np.ascontiguousarray(iota),
                "Wt": Wt,
                "brep": np.ascontiguousarray(brep),
                "ndst": np.ascontiguousarray(ndst[c]),
                "gcnt": np.ascontiguousarray(run_cnt[c : c + 1]),
            }
        )
    meta = dict(
        n_nodes=n_nodes, feat=feat, outd=outd, M=M, nt=nt, npc=npc,
        ninstr=run_cnt.shape[1],
        tile_runs=tile_runs, sli=sli, node=node, wcls=wcls,
        bias_is_zero=bool(np.all(b_arr == 0.0)),
    )
    return in_maps, meta


def kernel(inputs, W, b, src, dst):
    global LAST_RESULTS
    n_cores = 8
    inputs = np.asarray(inputs, dtype=F32)
    in_maps, meta = _prep(inputs, W, b, src, dst, n_cores)

    nc = _build_nc(
        meta["n_nodes"], meta["feat"], meta["outd"], meta["M"], meta["nt"],
        meta["npc"], meta["tile_runs"], n_cores, meta["wcls"],
        meta["bias_is_zero"], meta["ninstr"],
    )

    res = run_bass_kernel_spmd(
        nc,
        in_maps,
        core_ids=list(range(n_cores)),
        trace=bool(int(os.environ.get("KERNEL_TRACE", "0"))),
    )
    LAST_RESULTS = res

    # device output is node-major [npc, outd]; transpose to [outd, npc]
    out = np.stack(
        [r["out"].T for r in res.results], axis=0
    )  # [8, 64, 20000]
    return np.ascontiguousarray(out, dtype=F32)


# revision 8
# speedup vs baseline: 1.7832x; 1.0130x over previous
"""GraphSAGE/GraphConv (DGL norm='both') Bass kernel for 8 Trainium2 cores.

Math (reference):
  x[n,f]   : node features, n in [0,160000), f in [0,64)   (from inputs[8,64,20000])
  agg[d]   = norm_dst[d] * sum_{e: dst[e]=d} norm_src[src[e]] * x[src[e]]
  out      = leaky_relu(agg @ W + b, 0.01), returned as [8,64,20000] feature-major.

Device strategy (per core, vertex-cut on dst):
  - core c owns dst nodes [c*20000,(c+1)*20000) == output slice c of dim 0.
  - norm_src is folded into the x table on the host (x_tab[n] = x[n]*norm_src[n]),
    norm_dst is applied per dst node at the ZT evacuation (leaky-relu is
    positive-homogeneous, so the scale commutes through agg@W; bias handled
    before the activation).
  - edges bucketed by (dst tile of TW nodes, src window of WCLS rows), padded
    to 128-edge chunks.  Bulk dma_gather (int16 window-local indices, 256B fp32
    rows) pulls x[src] rows edge-major; descriptor generation is spread over
    4 SWDGE queues (distinct Q7 cpu pairs) so it runs ~3x parallel.
  - ACT casts gathered rows fp32->bf16; DVE builds the one-hot
    S01[e,d]=(dst_local[e]==d); TensorE accumulates aggT[f,d] += G[e,:]^T @ S01
    in PSUM per dst tile.
  - aggT evac (ACT, ->bf16), Z[d,o] = aggT[:,half]^T @ W per 128-row half
    (node-major!), evac with per-partition norm_dst scale + Lrelu (ACT),
    batched output DMA.  Host transposes [20000,64] -> [64,20000] per core.

Host does index/layout work plus the degree-normalization folds; all per-edge
feature math runs on device.
"""

import os

import numpy as np
import ml_dtypes

from concourse import bass, mybir
import concourse.bacc as bacc
from concourse.tile import TileContext
from concourse.bass_utils import run_bass_kernel_spmd

BF16 = ml_dtypes.bfloat16
F32 = np.float32

LAST_RESULTS = None  # test harness introspection (exec time / trace)

CHUNK = 128     # edges per matmul chunk (PE contraction dim)
TW = 256        # dst-tile width (one-hot columns / PSUM free dim)
WCLS = 32768    # src index window (int16 range for dma_gather)
MAX_RUN = 8     # chunks per dma_gather instruction (ring limit ~1024 idxs)
NQ = 4          # SWDGE queues (Q7 cpu pairs) to round-robin gathers over
STAGE_SLOTS = 8  # 128-row output halves per staged write-back
DLPAD = 300.0   # one-hot miss value for padded slots (> TW-1)


def _build_layout(src, dst, n_nodes, n_cores, npc, wcls, tw):
    """Tile-major slot layout: chunk m, partition p; arrays [128, M] per core,
    one shared shape.  For each dst tile t, runs of per-(t, r) segments
    (r = src window), each run <= MAX_RUN chunks.  Within each segment edges
    are sorted by src (better DRAM locality).
    """
    nt = -(-npc // tw)
    ncls = -(-n_nodes // wcls)

    owner = (dst // npc).astype(np.int64)
    rem = dst - owner * npc
    tile = rem // tw
    dl = rem - tile * tw
    r = (src // wcls).astype(np.int64)
    src_local = (src - r * wcls).astype(np.int16)

    # segment key: (core, tile, class); edges sorted by key then src
    key = (owner * nt + tile) * ncls + r
    order = np.lexsort((src, key))
    nseg = n_cores * nt * ncls
    counts = np.bincount(key, minlength=nseg).reshape(n_cores, nt, ncls)

    # shared chunk counts: max over cores
    chunks_tr = -(-counts.max(axis=0) // CHUNK)  # [nt, ncls]
    chunks_tr[0, 0] = max(chunks_tr[0, 0], 1)

    flat = chunks_tr.reshape(-1)
    seg_chunk_start = np.zeros(nt * ncls + 1, np.int64)
    np.cumsum(flat, out=seg_chunk_start[1:])
    M = int(seg_chunk_start[-1])

    # runs per tile: (a, b, r) chunk ranges, split at MAX_RUN
    tile_runs = []
    for t in range(nt):
        runs = []
        for rr in range(ncls):
            s = int(seg_chunk_start[t * ncls + rr])
            n = int(chunks_tr[t, rr])
            a = s
            while a < s + n:
                b = min(a + MAX_RUN, s + n)
                runs.append((a, b, rr))
                a = b
        tile_runs.append(runs)

    # slot fill (vectorized over all edges)
    seg_start = np.zeros(nseg + 1, np.int64)
    np.cumsum(counts.reshape(-1), out=seg_start[1:])
    key_s = key[order]
    pos = np.arange(len(order), dtype=np.int64) - seg_start[key_s]
    tr_key = key_s % (nt * ncls)            # (t, r) within core
    c_s = key_s // (nt * ncls)
    m_s = seg_chunk_start[tr_key] + pos // CHUNK
    p_s = pos - (pos // CHUNK) * CHUNK

    idx_slot = np.full((n_cores, 128, M), -1, np.int16)
    dl_all = np.full((n_cores, 128, M), DLPAD, BF16)
    idx_slot[c_s, p_s, m_s] = src_local[order]
    dl_all[c_s, p_s, m_s] = dl[order].astype(BF16)

    # per-core valid idx count per run (for num_idxs_reg; trailing -1s are
    # trimmed by the Q7 so descriptor work is exact per core)
    run_cnt = []
    for t in range(nt):
        for a, b, rr in tile_runs[t]:
            s0 = int(seg_chunk_start[t * ncls + rr])
            base = (a - s0) * CHUNK
            cnt = np.clip(counts[:, t, rr] - base, 0, (b - a) * CHUNK)
            run_cnt.append(cnt)
    run_cnt = np.stack(run_cnt, axis=1).astype(np.int32)  # [n_cores, ninstr]

    # dma_gather index stream: flat order i = chunk-major (k*128+p), wrapped
    # into 16 partitions (idx16[j, s] = flat[s*16+j]) and replicated x8.
    idx16 = np.zeros((n_cores, 128, M * (CHUNK // 16)), np.int16)
    for c in range(n_cores):
        flat_i = idx_slot[c].T.reshape(-1)            # [M*128], chunk-major
        wrapped = flat_i.reshape(-1, 16).T            # [16, M*8]
        idx16[c] = np.tile(wrapped, (8, 1))
    return idx16, dl_all, tile_runs, run_cnt, M, nt, ncls


def _build_nc(n_nodes, feat, outd, M, nt, npc, tile_runs, n_cores, wcls,
              bias_is_zero, ninstr):
    f32 = mybir.dt.float32
    i32 = mybir.dt.int32
    bf16 = mybir.dt.bfloat16
    i16 = mybir.dt.int16

    nc = bacc.Bacc(
        "TRN2",
        target_bir_lowering=False,
        debug=False,
        enable_asserts=False,
        num_devices=n_cores,
        num_swdge_queues=NQ,
    )

    nhalf = -(-npc // 128)
    scols = M * (CHUNK // 16)
    x_t = nc.dram_tensor("x_tab", [n_nodes, feat], f32, kind="ExternalInput")
    idx_t = nc.dram_tensor("idx16", [128, scols], i16, kind="ExternalInput")
    dl_t = nc.dram_tensor("dl", [128, M], bf16, kind="ExternalInput")
    iota_t = nc.dram_tensor("iota", [128, TW], bf16, kind="ExternalInput")
    W_t = nc.dram_tensor("Wt", [feat, outd], bf16, kind="ExternalInput")
    brep_t = nc.dram_tensor("brep", [128, outd], f32, kind="ExternalInput")
    ndst_t = nc.dram_tensor("ndst", [128, nhalf], f32, kind="ExternalInput")
    gcnt_t = nc.dram_tensor("gcnt", [1, ninstr], i32, kind="ExternalInput")
    out_t = nc.dram_tensor("out", [npc, outd], f32, kind="ExternalOutput")

    with TileContext(nc) as tc:
        with (
            tc.tile_pool(name="const", bufs=1) as constp,
            tc.tile_pool(name="gbuf", bufs=12) as gpool,
            tc.tile_pool(name="cast", bufs=6) as cpool,
            tc.tile_pool(name="onehot", bufs=6) as spool,
            tc.tile_pool(name="evac", bufs=3) as evacp,
            tc.tile_pool(name="stage", bufs=2) as stagep,
            tc.tile_pool(name="lk", bufs=2) as lkp,
            tc.tile_pool(name="psA", bufs=3, space="PSUM") as psA,
            tc.tile_pool(name="psZ", bufs=2, space="PSUM") as psZ,
        ):
            idx_sb = constp.tile([128, scols], i16)
            nc.sync.dma_start(idx_sb[:], idx_t[:])
            dl_sb = constp.tile([128, M], bf16)
            nc.sync.dma_start(dl_sb[:], dl_t[:])
            iota_sb = constp.tile([128, TW], bf16)
            nc.sync.dma_start(iota_sb[:], iota_t[:])
            W_sb = constp.tile([feat, outd], bf16)
            nc.sync.dma_start(W_sb[:], W_t[:])
            brep_sb = constp.tile([128, outd], f32)
            nc.sync.dma_start(brep_sb[:], brep_t[:])
            ndst_sb = constp.tile([128, nhalf], f32)
            nc.sync.dma_start(ndst_sb[:], ndst_t[:])
            gcnt_sb = constp.tile([1, ninstr], i32)
            nc.sync.dma_start(gcnt_sb[:], gcnt_t[:])
            NREG = 8
            LOOKAHEAD = 6
            cnt_regs = [
                nc.gpsimd.alloc_register(f"gcnt_reg{i}") for i in range(NREG)
            ]

            def load_cnt(i):
                if i < ninstr:
                    nc.gpsimd.reg_load(cnt_regs[i % NREG], gcnt_sb[0:1, i : i + 1])

            for i in range(LOOKAHEAD):
                load_cnt(i)

            for _ in range(12):
                g0 = gpool.tile([128, MAX_RUN * feat], f32, tag="g")
                nc.vector.memset(g0[:], 0.0)

            gq = 0            # gather instruction counter (queue round-robin)
            stage = None
            stage_fill = 0    # half-slots currently staged
            stage_base = 0    # first half index in the stage
            lrelu = mybir.ActivationFunctionType.Lrelu
            copyf = mybir.ActivationFunctionType.Copy

            def flush_stage():
                nonlocal stage, stage_fill, stage_base
                if stage is None or stage_fill == 0:
                    return
                ns = stage_fill
                span = stage[:, : ns * outd]
                if not bias_is_zero:
                    nc.vector.tensor_tensor(
                        out=span.rearrange("p (k f) -> p k f", f=outd),
                        in0=span.rearrange("p (k f) -> p k f", f=outd),
                        in1=brep_sb[:]
                        .rearrange("p (o f) -> p o f", o=1)
                        .to_broadcast([128, ns, outd]),
                        op=mybir.AluOpType.add,
                    )
                    zs = lkp.tile([128, STAGE_SLOTS * outd], f32, tag="zs")
                    nc.vector.tensor_scalar(
                        out=zs[:, : ns * outd],
                        in0=span,
                        scalar1=0.01,
                        scalar2=None,
                        op0=mybir.AluOpType.mult,
                    )
                    nc.vector.tensor_tensor(
                        out=span,
                        in0=span,
                        in1=zs[:, : ns * outd],
                        op=mybir.AluOpType.max,
                    )
                row0 = stage_base * 128
                nfull = min(ns, (npc - row0) // 128)
                if nfull > 0:
                    nc.sync.dma_start(
                        out_t[row0 : row0 + nfull * 128, :].rearrange(
                            "(t p) f -> p t f", p=128
                        ),
                        stage[:, : nfull * outd].rearrange(
                            "p (t f) -> p t f", f=outd
                        ),
                    )
                if nfull < ns:  # partial last 128-block (tail of the core)
                    rr0 = row0 + nfull * 128
                    nv = npc - rr0
                    nc.sync.dma_start(
                        out_t[rr0:npc, :],
                        stage[:nv, nfull * outd : (nfull + 1) * outd],
                    )
                stage = None
                stage_fill = 0

            for t in range(nt):
                runs = tile_runs[t]
                nch = sum(b - a for a, b, _ in runs)
                aggT = psA.tile([feat, TW], f32, tag="agg")
                j = 0
                for a, b, rr in runs:
                    K = b - a
                    row0 = rr * wcls
                    row1 = min(n_nodes, (rr + 1) * wcls)
                    gt = gpool.tile([128, MAX_RUN * feat], f32, tag="g")
                    nc.gpsimd.dma_gather(
                        out_ap=gt[:, : K * feat].rearrange(
                            "p (k f) -> p k f", f=feat
                        ),
                        in_ap=x_t[row0:row1, :],
                        idxs_ap=idx_sb[:, a * (CHUNK // 16) : b * (CHUNK // 16)],
                        num_idxs=K * CHUNK,
                        num_idxs_reg=cnt_regs[gq % NREG],
                        elem_size=feat,
                        queue_num=gq % NQ,
                    )
                    gq += 1
                    load_cnt(gq + LOOKAHEAD - 1)
                    ct = cpool.tile([128, MAX_RUN * feat], bf16, tag="c")
                    nc.scalar.activation(ct[:, : K * feat], gt[:, : K * feat], copyf)
                    st = spool.tile([128, MAX_RUN * TW], bf16, tag="s")
                    nc.vector.tensor_tensor(
                        out=st[:, : K * TW].rearrange("p (k d) -> p k d", d=TW),
                        in0=iota_sb[:]
                        .rearrange("p (o d) -> p o d", o=1)
                        .to_broadcast([128, K, TW]),
                        in1=dl_sb[:, a:b]
                        .rearrange("p (k o) -> p k o", o=1)
                        .to_broadcast([128, K, TW]),
                        op=mybir.AluOpType.is_equal,
                    )
                    for col in range(K):
                        nc.tensor.matmul(
                            out=aggT[:],
                            lhsT=ct[:, col * feat : (col + 1) * feat],
                            rhs=st[:, col * TW : (col + 1) * TW],
                            start=(j == 0),
                            stop=(j == nch - 1),
                        )
                        j += 1
                # aggT -> SBUF bf16, then per-128 half: Z[d,o], scale, stage
                aggs = evacp.tile([feat, TW], bf16, tag="ev")
                nc.scalar.activation(aggs[:], aggT[:], copyf)
                for h in range(TW // 128):
                    half = t * (TW // 128) + h
                    if half * 128 >= npc:
                        continue
                    if stage is None:
                        stage = stagep.tile([128, STAGE_SLOTS * outd], f32, tag="st")
                        stage_base = half
                    Z = psZ.tile([128, outd], f32, tag="z")
                    nc.tensor.matmul(
                        out=Z[:],
                        lhsT=aggs[:, h * 128 : (h + 1) * 128],
                        rhs=W_sb[:],
                        start=True,
                        stop=True,
                    )
                    sl = stage_fill
                    nc.scalar.activation(
                        stage[:, sl * outd : (sl + 1) * outd],
                        Z[:],
                        lrelu if bias_is_zero else copyf,
                        scale=ndst_sb[:, half : half + 1],
                        alpha=0.01,
                    )
                    stage_fill += 1
                    if stage_fill == STAGE_SLOTS:
                        flush_stage()
            flush_stage()

    nc.compile()
    return nc


def _prep(inputs, W, b, src, dst, n_cores, wcls=WCLS):
    sli, feat, node = inputs.shape
    n_nodes = sli * node
    outd = W.shape[1]
    npc = n_nodes // n_cores

    src = np.asarray(src).astype(np.int64)
    dst = np.asarray(dst).astype(np.int64)
    deg_out = np.bincount(src, minlength=n_nodes)
    deg_in = np.bincount(dst, minlength=n_nodes)
    norm_src = np.maximum(deg_out, 1).astype(F32) ** -0.5
    norm_dst = np.maximum(deg_in, 1).astype(F32) ** -0.5

    # node-major feature table with norm_src pre-folded
    x_tab = np.ascontiguousarray(
        np.asarray(inputs, dtype=F32).transpose(0, 2, 1).reshape(n_nodes, feat)
    ) * norm_src[:, None]
    x_tab = np.ascontiguousarray(x_tab, dtype=F32)

    idx16, dl_all, tile_runs, run_cnt, M, nt, ncls = _build_layout(
        src, dst, n_nodes, n_cores, npc, wcls, TW
    )

    nhalf = -(-npc // 128)
    iota = np.broadcast_to(np.arange(TW, dtype=F32), (128, TW)).astype(BF16)
    Wt = np.asarray(W, dtype=F32).astype(BF16)
    b_arr = np.asarray(b, dtype=F32).reshape(-1)
    brep = np.broadcast_to(b_arr, (128, outd)).astype(F32)
    ndst = np.ones((n_cores, 128, nhalf), F32)
    for c in range(n_cores):
        base = c * npc
        cols = np.arange(nhalf) * 128
        idxg = base + cols[None, :] + np.arange(128)[:, None]
        valid = idxg < base + npc
        ndst[c][valid] = norm_dst[idxg[valid]]

    in_maps = []
    for c in range(n_cores):
        in_maps.append(
            {
                "x_tab": x_tab,
                "idx16": np.ascontiguousarray(idx16[c]),
                "dl": np.ascontiguousarray(dl_all[c]),
                "iota": np.ascontiguousarray(iota),
                "Wt": Wt,
                "brep": np.ascontiguousarray(brep),
                "ndst": np.ascontiguousarray(ndst[c]),
                "gcnt": np.ascontiguousarray(run_cnt[c : c + 1]),
            }
        )
    meta = dict(
        n_nodes=n_nodes, feat=feat, outd=outd, M=M, nt=nt, npc=npc,
        ninstr=run_cnt.shape[1],
        tile_runs=tile_runs, sli=sli, node=node, wcls=wcls,
        bias_is_zero=bool(np.all(b_arr == 0.0)),
    )
    return in_maps, meta


def kernel(inputs, W, b, src, dst):
    global LAST_RESULTS
    n_cores = 8
    inputs = np.asarray(inputs, dtype=F32)
    in_maps, meta = _prep(inputs, W, b, src, dst, n_cores)

    nc = _build_nc(
        meta["n_nodes"], meta["feat"], meta["outd"], meta["M"], meta["nt"],
        meta["npc"], meta["tile_runs"], n_cores, meta["wcls"],
        meta["bias_is_zero"], meta["ninstr"],
    )

    res = run_bass_kernel_spmd(
        nc,
        in_maps,
        core_ids=list(range(n_cores)),
        trace=bool(int(os.environ.get("KERNEL_TRACE", "0"))),
    )
    LAST_RESULTS = res

    # device output is node-major [npc, outd]; transpose to [outd, npc]
    out = np.stack(
        [r["out"].T for r in res.results], axis=0
    )  # [8, 64, 20000]
    return np.ascontiguousarray(out, dtype=F32)
